# revision 27
# baseline (speedup 1.0000x reference)
"""Causal self-attention (RMS-normed QK + RoPE + v-mix) on 8 trn2 cores.

Sharding: tensor-parallel over heads x causal-balanced query split.
  - 12 heads -> 4 groups of 3 heads; group g runs on cores (2g, 2g+1).
  - Within a pair, core parity p owns the 8 query tiles with global tile
    index == p (mod 2) (128 rows each); attention runs over two 512-col
    query blocks, looping key tiles with causally-shrinking windows.
  - Each core emits a partial projection y_part = attn_g @ Wp[:,cg].T for
    its 1024 query rows; the host sums the 4 group partials per row.

Speed notes (vs v1):
  - all matmul operands are float32r end-to-end (dram+sbuf dtypes), 1
    PE cycle/row at moving>=256 instead of fp32's 4.
  - V projection runs in bf16 (stationary x-tiles, moving Wv), giving
    1 cycle/row at its 192-wide moving dim; error ~0.3% on half of v.
  - rms rsqrt + softmax denominators use the Abs_reciprocal_sqrt
    activation table (1/d = rsqrt(d)^2) -- no lane-starved DVE
    InstReciprocal, no Sqrt+recip chain.
  - attention is query-block-outer: per (head, block, k) one score
    matmul over the causally visible window, one exp, one PV matmul;
    diagonal/full masking via a single per-parity [128,256] atom table
    (k-parity picks the atom), applied to the window head.
  - softmax normalization deferred past the attention loop (frees the
    exp table), pv evacuated straight into packed at tiles (heads 0,1
    stacked 128-deep for the output projection; head1 partition-shifted
    via sbuf->sbuf DMA).
  - DMAs batched via 3D views (one descriptor per tensor/chunk).
"""

import sys

sys.path.insert(0, "/opt/trn_rl_repo")

import numpy as np

import concourse.bass as bass
from concourse import mybir
from concourse.tile import TileContext
from concourse.vector_clock import ScopedClock

F32 = mybir.dt.float32
F32R = mybir.dt.float32r
BF16 = mybir.dt.bfloat16
AF = mybir.ActivationFunctionType
ALU = mybir.AluOpType

T = 2048
D = 768
NH = 12
HD = 64
HPC = 3  # heads per core
C = HPC * HD  # 192 channels per group
NQ = 1024  # query rows per core
NKT = T // 128  # 16 key tiles
NDT = D // 128  # 6 contraction tiles
EPS = float(np.finfo(np.float32).eps)
MASKVAL = -1.0e30

# causal window sizes (in 128-tiles) per key tile, odd-parity structural
# bound; even-parity cores mask the window head via the atom table
COUNT0 = [4, 4, 3, 3, 2, 2, 1, 1]
COUNT1 = [4] * 10 + [3, 3, 2, 2, 1, 1]

TRACE = False
_CACHED = {}


def _patch_tile_tail():
    """walrus here rejects >1 sync-wait per instruction; TileContext's tail
    drain stacks one wait per active proc.  Spread them over wait_ge's."""
    if getattr(TileContext, "_tail_patched", False):
        return

    def _drain_and_barrier(self, tick_clock, wait_clock):
        nc = self.nc
        collector = nc.sync.nop()
        wait_clock.add_sem_waits(
            collector.ins, ScopedClock({None: tick_clock.global_clock})
        )
        si = collector.ins.sync_info
        waits = list(si.on_wait) if (si and si.on_wait) else []
        if len(waits) > 1:
            by_num = {h.num: h for h in wait_clock.sems.allocated().values()}
            kept, respawn = [], []
            for w in waits:
                if kept and w.id in by_num and w.wait_mode == "sem-ge-imm":
                    respawn.append(w)
                else:
                    kept.append(w)
            si.on_wait = kept
            for w in respawn:
                nc.sync.wait_ge(by_num[w.id], w.wait_value)
        nc.sync.drain()
        nc.all_engine_barrier()
        assert self.sems is not None
        popped = nc._tile_sem_poison_stack.pop()
        assert popped is self._sem_poison
        nc.clear_and_free_semaphores(list(self.sems.allocated().values()))
        nc.all_engine_barrier()

    TileContext._drain_and_barrier = _drain_and_barrier
    TileContext._tail_patched = True


def _split_multiwait_bir(bir_json):
    """Rewrite serialized BIR so no instruction carries more than one sync
    wait (this walrus build rejects >1): extra waits move onto single-wait
    NoOps inserted just before the instruction on the same engine."""
    import json as _json

    d = _json.loads(bir_json)
    n_split = 0
    for fn in d["functions"]:
        for bb in fn["blocks"]:
            out = []
            for inst in bb["instructions"]:
                si = inst.get("sync_info") or {}
                waits = si.get("on_wait") or []
                if len(waits) > 1:
                    for wi, w in enumerate(waits[:-1]):
                        n_split += 1
                        out.append(
                            {
                                "name": f"{inst['name']}-wsplit{wi}",
                                "opcode": "EventSemaphore",
                                "engine": inst["engine"],
                                "debug": inst.get("debug", 0),
                                "ins": [],
                                "outs": [],
                                "sync_info": {"on_update": [], "on_wait": [w]},
                            }
                        )
                    si["on_wait"] = [waits[-1]]
                out.append(inst)
            bb["instructions"] = out
    enc = _json.dumps(d)
    return enc.encode() if isinstance(bir_json, bytes) else enc


def _patch_wait_split():
    import concourse.bass_utils as bu
    import concourse.bass2jax as b2j

    if getattr(bu, "_wait_split_patched", False):
        return
    orig = bu.compile_bir_kernel

    def wrapped(bir_json, tmpdir, neff_name="file.neff"):
        return orig(_split_multiwait_bir(bir_json), tmpdir, neff_name=neff_name)

    bu.compile_bir_kernel = wrapped
    b2j.compile_bir_kernel = wrapped
    # let walrus double-buffer weight loads (default cmd disables it)
    orig_run = bu.run_command

    def run_patched(cmd, **kw):
        cmd = [
            c.replace("--enable-ldw-opt=false", "--enable-ldw-opt=false")
            for c in cmd
        ]
        return orig_run(cmd, **kw)

    bu.run_command = run_patched
    bu._wait_split_patched = True


def build_nc():
    _patch_tile_tail()
    _patch_wait_split()
    nc = bass.Bass("TRN2")

    xt = nc.dram_tensor("xt", [D, T], F32R, kind="ExternalInput")
    xq = nc.dram_tensor("xq", [D, NQ], F32R, kind="ExternalInput")
    xb = nc.dram_tensor("xb", [D, T], BF16, kind="ExternalInput")
    wq = nc.dram_tensor("wq", [D, C], F32R, kind="ExternalInput")
    wk = nc.dram_tensor("wk", [D, C], F32R, kind="ExternalInput")
    wv = nc.dram_tensor("wv", [128, NDT * C], BF16, kind="ExternalInput")
    wp = nc.dram_tensor("wp", [C, D], F32R, kind="ExternalInput")
    vin = nc.dram_tensor("vin", [128, NKT * C], F32, kind="ExternalInput")
    c4k = nc.dram_tensor("c4k", [128, T], F32R, kind="ExternalInput")
    s4k = nc.dram_tensor("s4k", [128, T], F32R, kind="ExternalInput")
    c4q = nc.dram_tensor("c4q", [128, NQ], F32R, kind="ExternalInput")
    s4q = nc.dram_tensor("s4q", [128, NQ], F32R, kind="ExternalInput")
    tri2 = nc.dram_tensor("tri2", [128, 256], F32, kind="ExternalInput")
    perm = nc.dram_tensor("perm", [128, 128], F32R, kind="ExternalInput")
    ones1 = nc.dram_tensor("ones1", [128, 66], F32R, kind="ExternalInput")
    sel3 = nc.dram_tensor("sel3", [3, 128], F32R, kind="ExternalInput")
    vones = nc.dram_tensor("vones", [128, 3 * NKT], F32R, kind="ExternalInput")
    yp = nc.dram_tensor("yp", [NQ, D], F32, kind="ExternalOutput")

    with TileContext(nc) as tc:
        with (
            tc.tile_pool(name="const", bufs=1) as constp,
            tc.tile_pool(name="persist", bufs=1) as pers,
        ):
            # ---- constants / tables (one DMA each) ----
            ones = constp.tile([128, 66], F32R, tag="ones")
            nc.scalar.dma_start(ones[:], ones1[:, :])
            sel3_sb = constp.tile([2, 128], F32R, tag="sel3")
            nc.scalar.dma_start(sel3_sb[:], sel3[0:2, :])
            selb_sb = constp.tile([1, 128], F32R, tag="selb")
            nc.scalar.dma_start(selb_sb[:], sel3[2:3, :])
            eps_sb = constp.tile([128, 1], F32, tag="eps")
            nc.vector.memset(eps_sb[:], EPS)
            c4k_sb = constp.tile([128, T], F32R, tag="c4k")
            s4k_sb = constp.tile([128, T], F32R, tag="s4k")
            c4q_sb = constp.tile([128, NQ], F32R, tag="c4q")
            s4q_sb = constp.tile([128, NQ], F32R, tag="s4q")
            tri2_sb = constp.tile([128, 256], F32, tag="tri2")
            perm_sb = constp.tile([128, 128], F32R, tag="perm")
            wp2_sb = constp.tile([128, D], F32R, tag="wp2")
            wp1_sb = constp.tile([64, D], F32R, tag="wp1")
            wq_sb = constp.tile([128, NDT * C], F32R, tag="wq")
            wk_sb = constp.tile([128, NDT * C], F32R, tag="wk")
            wv_sb = constp.tile([128, NDT * C], BF16, tag="wv")
            vin_sb = constp.tile([128, NKT * C], F32, tag="vin")
            VW0 = 3 * 65
            v_sb = pers.tile([128, NKT * VW0], F32R, tag="v")
            nc.gpsimd.dma_start(
                v_sb[:].rearrange("p (t c) -> p t c", t=3 * NKT)[:, :, 64:65],
                vones[:, :].rearrange("p (t c) -> p t c", c=1),
            )
            # early constants on the Activation queue, ordered by first use;
            # late tables ride the sync queue after the x chunks
            nc.scalar.dma_start(wq_sb[:], wq[:, :])
            nc.scalar.dma_start(wk_sb[:], wk[:, :])
            nc.scalar.dma_start(wv_sb[:], wv[:, :])
            nc.scalar.dma_start(vin_sb[:], vin[:, :])
            nc.gpsimd.dma_start(c4q_sb[:], c4q[:, :])
            nc.gpsimd.dma_start(s4q_sb[:], s4q[:, :])
            nc.gpsimd.dma_start(perm_sb[:], perm[:, :])
            nc.gpsimd.dma_start(c4k_sb[:], c4k[:, :])
            nc.gpsimd.dma_start(s4k_sb[:], s4k[:, :])

            # ---- persistent activations ----
            qA = pers.tile([128, NQ], F32R, tag="qA")
            qB = pers.tile([64, NQ], F32R, tag="qB")
            kA = pers.tile([128, T], F32R, tag="kA")
            kB = pers.tile([64, T], F32R, tag="kB")
            # v natural layout: per key tile, 3 heads x (64 dims + ones col)
            VW = 3 * 65
            # packed attention outputs: heads 0,1 on at2, head 2 on at1
            at2 = pers.tile([128, NQ], F32R, tag="at2")
            at1 = pers.tile([64, NQ], F32R, tag="at1")
            den = pers.tile([1, 6 * 512], F32, tag="den")

            # ============ phases A (projections) + B (rms/rope) ============
            with (
                tc.tile_pool(name="psA", bufs=3, space="PSUM") as psA,
                tc.tile_pool(name="psR", bufs=2, space="PSUM") as psR,
                tc.tile_pool(name="xtp", bufs=2) as xtp,
                tc.tile_pool(name="xbp", bufs=2) as xbp,
                tc.tile_pool(name="xqp", bufs=2) as xqp,
                tc.tile_pool(name="scrp", bufs=2) as scrp,
                tc.tile_pool(name="scr2", bufs=4) as scr2,
                tc.tile_pool(name="rowp", bufs=2) as rowp,
                tc.tile_pool(name="psrow", bufs=3, space="PSUM") as psrow,
            ):
                # Q projection, 2 chunks of 512 query columns
                for ch in range(NQ // 512):
                    c0 = 512 * ch
                    xq_ch = xqp.tile([128, NDT * 512], F32R, tag="xq")
                    nc.sync.dma_start(
                        xq_ch[:].rearrange("p (a c) -> p a c", a=NDT),
                        xq[:, c0 : c0 + 512].rearrange("(a p) c -> p a c", p=128),
                    )
                    for dst, m, coff in ((qA, 128, 0), (qB, 64, 128)):
                        ps = psA.tile([m, 512], F32, tag="psA", name="psq")
                        for d in range(NDT):
                            nc.tensor.matmul(
                                ps[:],
                                wq_sb[:, C * d + coff : C * d + coff + m],
                                xq_ch[:, 512 * d : 512 * (d + 1)],
                                start=(d == 0),
                                stop=(d == NDT - 1),
                            )
                        nc.vector.tensor_copy(dst[:, c0 : c0 + 512], ps[:])

                # K (f32r weight-stationary) and V (bf16 x-stationary)
                for ch in range(T // 512):
                    c0 = 512 * ch
                    xt_ch = xtp.tile([128, NDT * 512], F32R, tag="xt")
                    nc.sync.dma_start(
                        xt_ch[:].rearrange("p (a c) -> p a c", a=NDT),
                        xt[:, c0 : c0 + 512].rearrange("(a p) c -> p a c", p=128),
                    )
                    xb_ch = xbp.tile([128, NDT * 512], BF16, tag="xb")
                    nc.sync.dma_start(
                        xb_ch[:].rearrange("p (a c) -> p a c", a=NDT),
                        xb[:, c0 : c0 + 512].rearrange("(a p) c -> p a c", p=128),
                    )
                    for dst, m, coff in ((kA, 128, 0), (kB, 64, 128)):
                        ps = psA.tile([m, 512], F32, tag="psA", name="psk")
                        for d in range(NDT):
                            nc.tensor.matmul(
                                ps[:],
                                wk_sb[:, C * d + coff : C * d + coff + m],
                                xt_ch[:, 512 * d : 512 * (d + 1)],
                                start=(d == 0),
                                stop=(d == NDT - 1),
                            )
                        nc.vector.tensor_copy(dst[:, c0 : c0 + 512], ps[:])
                    for ti in range(4):
                        t = 4 * ch + ti
                        ps = psA.tile([128, 512], F32, tag="psA", name="psv")
                        for d in range(NDT):
                            nc.tensor.matmul(
                                ps[:, 0:C],
                                xb_ch[
                                    :,
                                    512 * d + 128 * ti : 512 * d + 128 * (ti + 1),
                                ],
                                wv_sb[:, C * d : C * (d + 1)],
                                start=(d == 0),
                                stop=(d == NDT - 1),
                            )
                        vt = v_sb[:, VW * t : VW * (t + 1)]
                        dst3 = vt.rearrange("p (h c) -> p h c", h=3)[:, :, 0:64]
                        src3 = ps[:, 0:C].rearrange("p (h c) -> p h c", h=3)
                        vin3 = vin_sb[:, C * t : C * (t + 1)].rearrange(
                            "p (h c) -> p h c", h=3
                        )
                        nc.vector.tensor_add(dst3, src3, vin3)

                # ---- phase B: rms norm + rope (normalize last) ----
                #   y = (raw*cos4 + swap(raw)*sinF4) * rb,  rb = rsqrt(ms+eps)

                def rms_rope(tA, tB, cos_sb, sin_sb, n_total):
                    for ch in range(n_total // 512):
                        c0 = 512 * ch
                        for tile_, P in ((tA, 128), (tB, 64)):
                            sl = tile_[:, c0 : c0 + 512]
                            sq = scrp.tile([128, 512], F32R, tag="sq", name="sq")
                            nc.vector.tensor_mul(sq[0:P, :], sl, sl)
                            rbp = psA.tile([128, 512], F32, tag="psA", name="rbp")
                            nr = P // 64  # stat rows (2 for A tiles, 1 for B)
                            ssq = psrow.tile([2, 512], F32, tag="row", name="ssq")
                            nc.tensor.matmul(
                                ssq[0:nr, :],
                                ones[0:P, 64 : 64 + nr],
                                sq[0:P, :],
                                start=True,
                                stop=True,
                            )
                            rln = rowp.tile([2, 512], F32, tag="rln")
                            nc.scalar.activation(
                                rln[0:nr, :],
                                ssq[0:nr, :],
                                AF.Ln,
                                bias=eps_sb[0:nr, :],
                                scale=1.0 / HD,
                            )
                            rrow = rowp.tile([2, 512], F32R, tag="rrow")
                            nc.scalar.activation(
                                rrow[0:nr, :], rln[0:nr, :], AF.Exp, scale=-0.5
                            )
                            nc.tensor.matmul(
                                rbp[0:P, :],
                                sel3_sb[0:nr, 0:P],
                                rrow[0:nr, :],
                                start=True,
                                stop=True,
                            )
                            qs = psR.tile([128, 512], F32, tag="psR", name="qs")
                            nc.tensor.matmul(
                                qs[0:P, :],
                                perm_sb[0:P, 0:P],
                                sl,
                                start=True,
                                stop=True,
                            )
                            m_ = scr2.tile([128, 512], F32R, tag="s2", name="m_")
                            t_ = scr2.tile([128, 512], F32R, tag="s2", name="t_")
                            nc.vector.tensor_mul(
                                m_[0:P, :], sl, cos_sb[0:P, c0 : c0 + 512]
                            )
                            nc.vector.tensor_mul(
                                t_[0:P, :], qs[0:P, :], sin_sb[0:P, c0 : c0 + 512]
                            )
                            u_ = scr2.tile([128, 512], F32R, tag="s2", name="u_")
                            nc.vector.tensor_add(
                                u_[0:P, :], m_[0:P, :], t_[0:P, :]
                            )
                            nc.vector.tensor_mul(sl, u_[0:P, :], rbp[0:P, :])

                rms_rope(qA, qB, c4q_sb, s4q_sb, NQ)
                rms_rope(kA, kB, c4k_sb, s4k_sb, T)

            # late tables: prefetch on the gpsimd swdge queue
            nc.gpsimd.dma_start(tri2_sb[:], tri2[:, :])
            nc.gpsimd.dma_start(wp2_sb[:], wp[0:128, :])
            nc.gpsimd.dma_start(wp1_sb[:], wp[128:192, :])

            # ================= phase C: attention =================
            with (
                tc.tile_pool(name="stp", bufs=3, space="PSUM") as stp,
                tc.tile_pool(name="pvp", bufs=2, space="PSUM") as pvp,
                tc.tile_pool(name="epool", bufs=4) as epool,
                tc.tile_pool(name="tmpp", bufs=1) as tmpp,
            ):
                tmp1 = tmpp.tile([64, NQ], F32R, tag="tmp1")
                for h in range(3):
                    kr = kA if h < 2 else kB
                    qr = qA if h < 2 else qB
                    poff = 64 * (h % 2)
                    for blk in range(2):
                        q0 = 512 * blk
                        counts = COUNT0 if blk == 0 else COUNT1
                        pv = pvp.tile([65, 512], F32, tag="pv", name="pv")
                        npair = len(counts) // 2
                        LOOK = 2  # pairs of score/exp issued ahead of their pv

                        def emit_pv(pj, ets):
                            ki2 = 2 * pj
                            cnt2 = counts[ki2]
                            w02 = (4 - cnt2) * 128
                            ap2 = cnt2 * 128
                            o2 = ap2 if 2 * ap2 <= 512 or ap2 == 512 else 512
                            et2 = ets[pj]
                            nc.tensor.matmul(
                                pv[:, w02 : w02 + ap2],
                                v_sb[
                                    :,
                                    VW * ki2 + 65 * h : VW * ki2 + 65 * (h + 1),
                                ],
                                et2[:, 0:ap2],
                                start=(pj == 0),
                                stop=False,
                                skip_group_check=True,
                            )
                            nc.tensor.matmul(
                                pv[:, w02 : w02 + ap2],
                                v_sb[
                                    :,
                                    VW * (ki2 + 1)
                                    + 65 * h : VW * (ki2 + 1)
                                    + 65 * (h + 1),
                                ],
                                et2[:, o2 : o2 + ap2],
                                start=False,
                                stop=(pj == npair - 1),
                                skip_group_check=True,
                            )

                        ets = {}
                        for pi in range(npair):
                            ki = 2 * pi
                            cnt = counts[ki]  # == counts[ki+1]
                            w0 = (4 - cnt) * 128
                            ap = cnt * 128
                            # second window at off2: bank-aligned when needed
                            off2 = ap if 2 * ap <= 512 or ap == 512 else 512
                            st = stp.tile([128, 1024], F32, tag="st", name="st")
                            nc.tensor.matmul(
                                st[:, 0:ap],
                                kr[poff : poff + 64, 128 * ki : 128 * (ki + 1)],
                                qr[poff : poff + 64, q0 + w0 : q0 + w0 + ap],
                                start=True,
                                stop=True,
                            )
                            nc.tensor.matmul(
                                st[:, off2 : off2 + ap],
                                kr[
                                    poff : poff + 64,
                                    128 * (ki + 1) : 128 * (ki + 2),
                                ],
                                qr[poff : poff + 64, q0 + w0 : q0 + w0 + ap],
                                start=True,
                                stop=True,
                            )
                            if blk == 0 or ki >= 8:
                                # one strided add masks both window heads
                                nc.vector.tensor_add(
                                    st[:, 0 : 2 * off2].rearrange(
                                        "p (a c) -> p a c", a=2
                                    )[:, :, 0:128],
                                    st[:, 0 : 2 * off2].rearrange(
                                        "p (a c) -> p a c", a=2
                                    )[:, :, 0:128],
                                    tri2_sb[:].rearrange(
                                        "p (a c) -> p a c", a=2
                                    ),
                                )
                            et = epool.tile(
                                [128, 1024], F32R, tag="e", name="et"
                            )
                            nc.scalar.activation(
                                et[:, 0 : off2 + ap], st[:, 0 : off2 + ap], AF.Exp
                            )
                            ets[pi] = et
                            if pi >= LOOK:
                                emit_pv(pi - LOOK, ets)
                        for pj in range(max(0, npair - LOOK), npair):
                            emit_pv(pj, ets)
                        # evacuate raw pv + denominator (normalize later)
                        atdst = at2 if h == 0 else (tmp1 if h == 1 else at1)
                        nc.vector.tensor_copy(
                            atdst[0:64, q0 : q0 + 512], pv[0:64, :]
                        )
                        nc.vector.tensor_copy(
                            den[:, 512 * (2 * h + blk) : 512 * (2 * h + blk + 1)],
                            pv[64:65, :],
                        )
                # head 1 into at2's upper partitions (partition shift => DMA)
                nc.sync.dma_start(at2[64:128, :], tmp1[0:64, :])

            # ============ phase C': deferred softmax normalization ============
            with (
                tc.tile_pool(name="drp", bufs=2) as drp,
                tc.tile_pool(name="psn", bufs=2, space="PSUM") as psn,
            ):
                drb = drp.tile([1, 6 * 512], F32, tag="drb")
                nc.scalar.activation(drb[:], den[:], AF.Ln)
                drb2 = drp.tile([1, 6 * 512], F32R, tag="drb2")
                nc.scalar.activation(drb2[:], drb[:], AF.Exp, scale=-1.0)
                for h in range(3):
                    for blk in range(2):
                        q0 = 512 * blk
                        r0 = 512 * (2 * h + blk)
                        po = 64 if h == 1 else 0
                        at = at2 if h < 2 else at1
                        rbn = psn.tile([128, 512], F32, tag="rbn", name="rbn")
                        if po == 0:
                            nc.tensor.matmul(
                                rbn[0:64, :],
                                ones[0:1, 0:64],
                                drb2[:, r0 : r0 + 512],
                                start=True,
                                stop=True,
                                tile_position=(0, 0),
                            )
                        else:
                            # broadcast to all 128 partitions, use the top half
                            nc.tensor.matmul(
                                rbn[0:128, :],
                                selb_sb[0:1, :],
                                drb2[:, r0 : r0 + 512],
                                start=True,
                                stop=True,
                            )
                        nc.vector.tensor_mul(
                            at[po : po + 64, q0 : q0 + 512],
                            at[po : po + 64, q0 : q0 + 512],
                            rbn[po : po + 64, :],
                        )

            # ================= phase D: output projection =================
            with (
                tc.tile_pool(name="psy", bufs=2, space="PSUM") as psy,
                tc.tile_pool(name="ypool", bufs=2) as ypool,
            ):
                for j in range(8):
                    ps = psy.tile([128, D], F32, tag="psy", name="psy")
                    for n0, n1 in ((0, 512), (512, D)):
                        nc.tensor.matmul(
                            ps[:, n0:n1],
                            at2[:, 128 * j : 128 * (j + 1)],
                            wp2_sb[:, n0:n1],
                            start=True,
                            stop=False,
                        )
                        nc.tensor.matmul(
                            ps[:, n0:n1],
                            at1[:, 128 * j : 128 * (j + 1)],
                            wp1_sb[:, n0:n1],
                            start=False,
                            stop=True,
                        )
                    yt = ypool.tile([128, D], F32, tag="y")
                    nc.vector.tensor_copy(yt[:], ps[:])
                    nc.sync.dma_start(yp[128 * j : 128 * (j + 1), :], yt[:])

    return nc


def _host_prep(x, vi, Wq, Wk, Wv, Wp, lamb):
    import ml_dtypes

    lam = float(lamb)
    xtf = np.ascontiguousarray(x[0].T, dtype=np.float32)  # [768, 2048]

    inv_freq = (1.0 / 10000.0) ** (np.arange(0, HD, 2, dtype=np.float32) / HD)
    tpos = np.arange(T, dtype=np.float32)
    freqs = np.outer(tpos, inv_freq).astype(np.float32)  # [T, 32]
    cosT = np.cos(freqs).T.astype(np.float32)  # [32, T]
    sinT = np.sin(freqs).T.astype(np.float32)
    c4 = np.ascontiguousarray(np.vstack([cosT] * 4))  # [128, T]
    # signed sin stack: y = raw*cos4 + swap(raw)*sinF4
    s4 = np.ascontiguousarray(np.vstack([sinT, -sinT, sinT, -sinT]))
    scale = float(1.0 / np.sqrt(np.float32(HD)))
    # block-diag half-swap: qs = permf.T @ q swaps rows [0:32]<->[32:64]
    permf = np.zeros((128, 128), dtype=np.float32)
    for b in range(2):
        for i in range(32):
            permf[64 * b + 32 + i, 64 * b + i] = 1.0
            permf[64 * b + i, 64 * b + 32 + i] = 1.0

    tri = np.where(
        np.arange(128)[None, :] >= np.arange(128)[:, None], 0.0, MASKVAL
    ).astype(np.float32)  # [p=key-in-tile, c=query-in-tile]
    full = np.full((128, 128), MASKVAL, dtype=np.float32)
    zero = np.zeros((128, 128), dtype=np.float32)

    qcols_by_par = {}
    for par in (0, 1):
        jj = np.arange(8)
        qcols_by_par[par] = (
            256 * jj[:, None] + 128 * par + np.arange(128)[None, :]
        ).reshape(-1)

    xbf = xtf.astype(ml_dtypes.bfloat16)

    def chunk_major(xm, nch):
        # [(a p), (ch c)] -> [p, (ch a c)] with a=NDT, c=512
        a = xm.reshape(NDT, 128, nch, 512)
        return np.ascontiguousarray(
            a.transpose(1, 2, 0, 3).reshape(128, nch * NDT * 512)
        )

    def part_major(wm, blocks, cols):
        # [(a p), c] -> [p, (a c)]
        return np.ascontiguousarray(
            wm.reshape(blocks, 128, cols).transpose(1, 0, 2).reshape(128, -1)
        )

    # ones + stat-row selector columns; broadcast selector rows
    ones1m = np.ones((128, 66), dtype=np.float32)
    ones1m[:, 64] = (np.arange(128) < 64).astype(np.float32)
    ones1m[:, 65] = (np.arange(128) >= 64).astype(np.float32)
    sel3m = np.zeros((3, 128), dtype=np.float32)
    sel3m[0, 0:64] = 1.0
    sel3m[1, 64:128] = 1.0
    sel3m[2, :] = 1.0

    in_maps = []
    for core in range(8):
        g, par = core // 2, core % 2
        cg = slice(C * g, C * (g + 1))
        qcols = qcols_by_par[par]
        # mask atoms: [even-k atom | odd-k atom] for this parity
        if par == 0:
            tri2m = np.hstack([tri, full])
        else:
            tri2m = np.hstack([zero, tri])
        in_maps.append(
            {
                "xt": xtf,
                "xq": np.ascontiguousarray(xtf[:, qcols]),
                "xb": xbf,
                "wq": np.ascontiguousarray(Wq[cg, :].T),
                "wk": np.ascontiguousarray(Wk[cg, :].T),
                "wv": part_major(((1.0 - lam) * Wv[cg, :].T).astype(ml_dtypes.bfloat16), NDT, C),
                "wp": np.ascontiguousarray(Wp[:, cg].T),
                "vin": part_major((lam * vi[0][:, cg]).astype(np.float32), NKT, C),
                "c4k": c4,
                "s4k": s4,
                "c4q": np.ascontiguousarray(scale * c4[:, qcols]),
                "s4q": np.ascontiguousarray(scale * s4[:, qcols]),
                "tri2": np.ascontiguousarray(tri2m),
                "perm": permf,
                "ones1": ones1m,
                "sel3": sel3m,
                "vones": np.ones((128, 3 * NKT), dtype=np.float32),
            }
        )
    return in_maps, qcols_by_par


def kernel(x, vi, Wq, Wk, Wv, Wp, lamb):
    from concourse.bass_utils import run_bass_kernel_spmd

    x = np.asarray(x, dtype=np.float32)
    vi = np.asarray(vi, dtype=np.float32)
    Wq = np.asarray(Wq, dtype=np.float32)
    Wk = np.asarray(Wk, dtype=np.float32)
    Wv = np.asarray(Wv, dtype=np.float32)
    Wp = np.asarray(Wp, dtype=np.float32)

    in_maps, qcols_by_par = _host_prep(x, vi, Wq, Wk, Wv, Wp, lamb)
    if "nc" not in _CACHED:
        _CACHED["nc"] = build_nc()
    nc = _CACHED["nc"]
    res = run_bass_kernel_spmd(
        nc, in_maps, core_ids=list(range(8)), trace=TRACE
    )
    _CACHED["last_result"] = res

    y = np.zeros((T, D), dtype=np.float32)
    for core in range(8):
        y[qcols_by_par[core % 2]] += res.results[core]["yp"]
    return y[None]
